# revision 25
# baseline (speedup 1.0000x reference)
"""Causal single-head attention on 8 Trainium2 NeuronCores.

Problem: x [4, 2048, 1024] f32; Wq/Wk/Wv [1024, 1024] f32.
  q,k,v = x@W*; out = softmax(causal(q k^T / sqrt(d))) @ v.

Sharding: 8 cores = 4 batches x 2 query-halves. Causal load balancing via
"fold" assignment of 512-query blocks: core (b, 0) takes query blocks
{3, 0} of its batch, core (b, 1) takes {2, 1}. Each core owns two
512-query "supers" whose key-prefix lengths are padded to the
compile-time slot shape (16, 8) x 128-key tiles; the pad region is
neutralized by an additive -60000 mask (host-built, tiny) so a single
program serves all cores (SPMD). Only the 16 tile-slots that are masked
on at least one core get the mask-add (in-place in PSUM); the other 8
exp straight out of the matmul accumulator.

The k/v projections are split across each core pair: every core projects
only its half of the batch tokens and the halves are exchanged with a
pair-wise AllGather through DRAM bounce buffers. Schedule: kproj ->
start kT gather -> vproj -> start v gather -> qproj s0 -> qproj s1 ->
scores s0 -> scores s1 -> AV s1 -> AV s0, so both collectives are
covered by ~55us of projection work and the PE never waits on them.
AV does the small super first: its 8 kt tiles only read v rows 0..1023
(the first AllGather half), buying time for the second half to land.

Post-collective redistribution is spread across the sync and gpsimd DMA
queues (a single queue serializes the chunks: the first one would
complete ~18us after issue instead of ~5), and the v collective is held
back until the kT chunks have drained (witness copies gate v_in's
staging DMA): collective fabric traffic starves local DMA rings, so any
chunk still in flight when a collective starts finishes only after it.

Precision: projections and att@v run in fp16 (1 PE cycle/row; fp32 is
4x slower); qT/kT are quantized to fp8e4m3 after the fp16 projection
and the scores matmuls use fp8 DoubleRow perf mode (2 contraction
subtiles per pass, ~1.8x fp16 throughput, and it halves the kT
exchange/redistribution bytes). Measured end-to-end rel err 1.18e-2
against the 2e-2 budget; accumulation stays fp32 in PSUM throughout.
Output is fp16, converted to f32 on the host (~1e-4 additional).

SBUF is tight, so the exp(scores) tiles alias scratch that is dead by
then: e(super 0) lives in the two xkv staging tiles and e(super 1) in
the vproj staging tile.

Per-core dataflow (all big matmuls N=512):
  kT-half[e, tok] fp8, v-half[tok, e] fp16 -> AllGather within pair
  qT[e, q] = Wq^T x_q^T fp16 -> fp8        (per super)
  scores S^T[k, q] = kT-block^T qT         (fp8 DoubleRow, 4 passes)
  E = exp((S^T [+ mask]) / 32)             (ACT, fp16 out)
  out[q, e] = (E^T v) / (E^T 1)            (denominator via N=2
                                            ones-matmul, fp16)

Softmax max-subtraction is skipped deliberately: logits*scale are
bounded (|s|/32 < ~2.5), so exp is well-conditioned.
"""

import os
import sys

sys.path.insert(0, "/opt/trn_rl_repo")

from contextlib import ExitStack

import numpy as np

import concourse.bass as bass  # noqa: F401
import concourse.tile as tile
from concourse import bacc, mybir
from concourse.bass_utils import run_bass_kernel_spmd

B, T, D = 4, 2048, 1024
P = 128                 # partitions
DC = D // P             # 8 contraction chunks
QSUP = 512              # queries per super
NSUP = 2                # supers per core
NQ = QSUP * NSUP        # 1024 queries per core
SLOT_KT = (16, 8)       # 128-key tiles per super slot (compile-time, all cores)
NKT = sum(SLOT_KT)      # 24
HT = T // 2             # 1024 tokens projected per core (half of the pair)
HCH = HT // P           # 8 token chunks per half
TCH = T // P            # 16 key/value 128-token chunks
SCALE = 1.0 / 32.0      # 1/sqrt(D)
MASK_NEG = -60000.0     # representable in fp16; exp((s+m)/32) == 0

# tile-slots that get the additive mask (union over both cores of a pair):
# slot A (16 tiles): tiles 8..15; slot B (8 tiles): tiles 0..7.
MASKED = [(0, kt) for kt in range(8, 16)] + [(1, kt) for kt in range(8)]
NMSK = len(MASKED)      # 16

F16 = mybir.dt.float16
F32 = mybir.dt.float32
F8 = mybir.dt.float8e4

_CACHE = {}

last_exec_time_ns = None  # set when BASS_KERNEL_TRACE=1


def _build_program():
    nc = bacc.Bacc("TRN2", target_bir_lowering=False, debug=False, num_devices=8)

    xq_d = nc.dram_tensor("xq", [NSUP, P, DC, QSUP], F16, kind="ExternalInput")
    xkv_d = nc.dram_tensor("xkv", [2, P, DC, QSUP], F16, kind="ExternalInput")
    wq_d = nc.dram_tensor("wq", [P, DC, DC, P], F16, kind="ExternalInput")
    wk_d = nc.dram_tensor("wk", [P, DC, DC, P], F16, kind="ExternalInput")
    wv_d = nc.dram_tensor("wv", [P, DC, D], F16, kind="ExternalInput")
    msk_d = nc.dram_tensor("msk", [P, NMSK, QSUP], F16, kind="ExternalInput")
    out_d = nc.dram_tensor("out", [NQ, D], F16, kind="ExternalOutput")

    with tile.TileContext(nc) as tc, ExitStack() as stack:
        p_wq = stack.enter_context(tc.tile_pool(name="wq", bufs=1))
        p_xq = stack.enter_context(tc.tile_pool(name="xq", bufs=1))
        p_kt = stack.enter_context(tc.tile_pool(name="kt", bufs=1))
        p_v = stack.enter_context(tc.tile_pool(name="v", bufs=1))
        p_qt = stack.enter_context(tc.tile_pool(name="qt", bufs=1))
        p_misc = stack.enter_context(tc.tile_pool(name="misc", bufs=1))
        p_dram = stack.enter_context(tc.tile_pool(name="dram", bufs=1, space="DRAM"))
        p_wk = stack.enter_context(tc.tile_pool(name="wk", bufs=1))
        p_wv = stack.enter_context(tc.tile_pool(name="wv", bufs=1))
        p_xkv = stack.enter_context(tc.tile_pool(name="xkv", bufs=1))
        p_half = stack.enter_context(tc.tile_pool(name="half", bufs=1))
        p_m = stack.enter_context(tc.tile_pool(name="msk", bufs=1))
        p_out = stack.enter_context(tc.tile_pool(name="outp", bufs=2))
        ps512 = stack.enter_context(tc.tile_pool(name="ps512", bufs=3, space="PSUM"))
        psav = stack.enter_context(tc.tile_pool(name="psav", bufs=2, space="PSUM"))
        psd = stack.enter_context(tc.tile_pool(name="psd", bufs=1, space="PSUM"))

        # ---- persistent tensors ----
        kt_t = p_kt.tile([P, DC, T], F8)            # k^T  [e, tok] fp8
        v_t = p_v.tile([P, TCH, D], F16)            # v    [tok, e]
        # staging scratch shared by kproj ([P, ec, tok]) then vproj
        # ([P, tok-chunk, e]) then e(super 1); all roles fit [P, 8, 1024]
        half_t = p_half.tile([P, 8, 1024], F16)
        kt8_t = p_half.tile([P, DC, HT], F8, tag="kt8")   # kproj fp8 staging
        msk_t = p_m.tile([P, NMSK, QSUP], F16)

        ones_t = p_misc.tile([P, 2], F16, tag="ones")
        nc.gpsimd.memset(ones_t[:], 1.0)

        # ---- need-ordered input loads ----
        # the first kproj accumulation group needs wk[0] + xkv[0]: fan
        # those across all three DMA queues so they land in parallel,
        # then stream the rest as few large transfers as possible
        w0 = p_wk.tile([P, DC, P], F16, tag="wk0")
        nc.sync.dma_start(w0[:], wk_d.ap()[:, 0])
        xc0 = p_xkv.tile([P, DC, QSUP], F16, tag="xkv0")
        nc.scalar.dma_start(xc0[:, 0:4, :], xkv_d.ap()[0][:, 0:4, :])
        nc.gpsimd.dma_start(xc0[:, 4:8, :], xkv_d.ap()[0][:, 4:8, :])
        xkv_c = [xc0]
        wk_all = p_wk.tile([P, DC - 1, DC, P], F16, tag="wkall")
        for ec in range(1, DC):
            nc.sync.dma_start(wk_all[:, ec - 1], wk_d.ap()[:, ec])
        wk_c = [w0] + [wk_all[:, ec - 1] for ec in range(1, DC)]
        xc1 = p_xkv.tile([P, DC, QSUP], F16, tag="xkv1")
        nc.gpsimd.dma_start(xc1[:], xkv_d.ap()[1])
        xkv_c.append(xc1)
        # wv alone behind xkv0a on the scalar ring (vproj needs it ~50us
        # in); the bulky q-side transfers ride sync behind the wk stream
        wv_t = p_wv.tile([P, DC, D], F16)
        nc.scalar.dma_start(wv_t[:], wv_d.ap())
        xq_tiles = []
        xq_q = [nc.sync, nc.gpsimd]   # super 1 inputs ride the gpsimd ring
        for s in range(NSUP):
            xq_t = p_xq.tile([P, DC, QSUP], F16, tag=f"xq{s}")
            for dc in range(DC):
                xq_q[s].dma_start(xq_t[:, dc], xq_d.ap()[s][:, dc])
            xq_tiles.append([xq_t[:, dc] for dc in range(DC)])
        wq_all = p_wq.tile([P, DC, DC, P], F16, tag="wqall")
        for ec in range(DC):
            nc.gpsimd.dma_start(wq_all[:, ec], wq_d.ap()[:, ec])
        wq_tiles = [wq_all[:, ec] for ec in range(DC)]
        nc.gpsimd.dma_start(msk_t[:], msk_d.ap())
        # warmup collective: establishes the CC channel and rank sync
        # well before the kT gather, without blocking the input loads
        # queued ahead of it
        warm_in = p_dram.tile([P, 2], F16, tag="warm_in")
        warm_out = p_dram.tile([2, P, 2], F16, tag="warm_out")
        nc.gpsimd.dma_start(warm_in[:], ones_t[:])
        nc.gpsimd.collective_compute(
            "AllGather", mybir.AluOpType.bypass,
            replica_groups=[[0, 1], [2, 3], [4, 5], [6, 7]],
            ins=[warm_in.opt()], outs=[warm_out.opt()])

        # ---- P1a: kT for own half -> pair AllGather ----
        # ec-outer: each wk chunk feeds two accumulation groups before the
        # next chunk is needed, doubling the DMA deadline per chunk
        kt_in = p_dram.tile([P, DC, HT], F8, tag="kt_in")
        kt_out = p_dram.tile([2, P, DC, HT], F8, tag="kt_out")
        for ec in range(DC):
            for kt2 in range(HT // QSUP):
                acc = ps512.tile([P, QSUP], F32, tag="ps512")
                for dc in range(DC):
                    nc.tensor.matmul(
                        acc[:], wk_c[ec][:, dc, :],
                        xkv_c[kt2][:, dc, :],
                        start=(dc == 0), stop=(dc == DC - 1))
                nc.scalar.copy(
                    kt8_t[:, ec, kt2 * QSUP:(kt2 + 1) * QSUP], acc[:])
            if ec == DC // 2 - 1:
                nc.sync.dma_start(kt_in[:, 0:DC // 2, :],
                                  kt8_t[:, 0:DC // 2, :])
        nc.sync.dma_start(kt_in[:, DC // 2:DC, :], kt8_t[:, DC // 2:DC, :])
        nc.gpsimd.collective_compute(
            "AllGather", mybir.AluOpType.bypass,
            replica_groups=[[0, 1], [2, 3], [4, 5], [6, 7]],
            ins=[kt_in.opt()], outs=[kt_out.opt()])

        # ---- P1b: v for own half -> pair AllGather ----
        for tk in range(HCH):
            for eh in range(2):
                acc = ps512.tile([P, QSUP], F32, tag="ps512")
                for dc in range(DC):
                    nc.tensor.matmul(
                        acc[:],
                        xkv_c[tk // 4][:, dc, (tk % 4) * P:(tk % 4 + 1) * P],
                        wv_t[:, dc, eh * QSUP:(eh + 1) * QSUP],
                        start=(dc == 0), stop=(dc == DC - 1))
                nc.vector.tensor_copy(
                    half_t[:, tk, eh * QSUP:(eh + 1) * QSUP], acc[:])
        v_in = p_dram.tile([P, HCH, D], F16, tag="v_in")
        v_out = p_dram.tile([2, P, HCH, D], F16, tag="v_out")

        # ---- kT distribution: 512-key chunks spread over sync+gpsimd ----
        dist_q = [nc.sync, nc.sync, nc.gpsimd, nc.gpsimd]
        for h in range(2):
            for c2 in range(2):
                dist_q[h * 2 + c2].dma_start(
                    kt_t[:, :, h * HT + c2 * QSUP:h * HT + (c2 + 1) * QSUP],
                    kt_out[h][:, :, c2 * QSUP:(c2 + 1) * QSUP])
        # hold the v collective until ALL FOUR kt distribution chunks have
        # drained: collective fabric traffic starves the local DMA rings,
        # so any chunk still in flight when it starts completes only after
        # it ends. The CC-core pulls collectives as soon as their INPUT is
        # ready (queue order alone cannot delay it), so v_in's staging DMA
        # itself is gated behind per-chunk witness copies.
        kt_rdy = p_misc.tile([P, 4], F16, tag="kt_rdy")
        for c4 in range(4):
            nc.gpsimd.tensor_copy(
                kt_rdy[:, c4:c4 + 1],
                kt_t[:, DC - 1, (c4 + 1) * QSUP - 1:(c4 + 1) * QSUP])
        nc.gpsimd.dma_start(v_in[:], half_t[:])

        nc.gpsimd.collective_compute(
            "AllGather", mybir.AluOpType.bypass,
            replica_groups=[[0, 1], [2, 3], [4, 5], [6, 7]],
            ins=[v_in.opt()], outs=[v_out.opt()])
        for h in range(2):
            for c2 in range(2):
                dist_q[h * 2 + c2].dma_start(
                    v_t[:, h * HCH + c2 * 4:h * HCH + (c2 + 1) * 4, :],
                    v_out[h][:, c2 * 4:(c2 + 1) * 4, :])

        # ---- q projections (cover the AllGathers) ----
        qt_s = []
        for s in range(NSUP):
            xq_c = xq_tiles[s]
            qt_t = p_qt.tile([P, DC, QSUP], F8, tag=f"qt{s}")
            for ec in range(DC):
                acc = ps512.tile([P, QSUP], F32, tag="ps512")
                for dc in range(DC):
                    nc.tensor.matmul(acc[:], wq_tiles[ec][:, dc, :],
                                     xq_c[dc][:],
                                     start=(dc == 0), stop=(dc == DC - 1))
                nc.scalar.copy(qt_t[:, ec, :], acc[:])
            qt_s.append(qt_t)

        # ---- P2a: scores + exp for both supers ----
        # e(super 0) aliases the dead xkv staging tiles, e(super 1) the
        # kproj/vproj scratch
        def e_slice(s, kt, cols=slice(0, QSUP)):
            if s == 0:
                return xkv_c[kt // DC][:, kt % DC, cols]
            return half_t[:, kt, cols]

        mask_i = {(s, kt): i for i, (s, kt) in enumerate(MASKED)}
        for s in range(NSUP):
            nkt = SLOT_KT[s]
            qt = qt_s[s]
            for kt in range(nkt):
                acc = ps512.tile([P, QSUP], F32, tag="ps512")
                for e2 in range(DC // 2):
                    nc.tensor.matmul(
                        acc[:],
                        kt_t[:, 2 * e2:2 * e2 + 2, kt * P:(kt + 1) * P],
                        qt[:, 2 * e2:2 * e2 + 2, :],
                        start=(e2 == 0), stop=(e2 == DC // 2 - 1),
                        perf_mode=mybir.MatmulPerfMode.DoubleRow)
                mi = mask_i.get((s, kt))
                if mi is not None:
                    nc.vector.tensor_add(acc[:], acc[:], msk_t[:, mi, :])
                nc.scalar.activation(e_slice(s, kt), acc[:],
                                     mybir.ActivationFunctionType.Exp,
                                     scale=SCALE)

        # ---- P2b: att @ v, normalize, store ----
        d_acc = psd.tile([P, 4, 2], F32)
        # super 1 first: its 8 kt tiles only touch v rows 0..1023, which
        # arrive in the first v_out half; super 0's tail tiles need the
        # second half, which lands while super 1 is being computed
        for s in (1, 0):
            nkt = SLOT_KT[s]
            for qs in range(4):
                o_acc = psav.tile([P, D], F32, tag="av")
                for kt in range(nkt):
                    lhs = e_slice(s, kt, slice(qs * P, (qs + 1) * P))
                    nc.tensor.matmul(o_acc[:, 0:QSUP], lhs,
                                     v_t[:, kt, 0:QSUP],
                                     start=(kt == 0),
                                     stop=(kt == nkt - 1))
                    nc.tensor.matmul(o_acc[:, QSUP:D], lhs,
                                     v_t[:, kt, QSUP:D],
                                     start=(kt == 0),
                                     stop=(kt == nkt - 1))
                    nc.tensor.matmul(d_acc[:, qs, :], lhs, ones_t[:],
                                     start=(kt == 0),
                                     stop=(kt == nkt - 1))
                dinv = p_misc.tile([P, 1], F32, tag=f"dinv{s}{qs}")
                nc.vector.reciprocal(dinv[:], d_acc[:, qs, 0:1])
                o_t = p_out.tile([P, D], F16, tag="o")
                nc.vector.tensor_scalar_mul(o_t[:], o_acc[:], dinv[:])
                row = s * QSUP + qs * P
                nc.sync.dma_start(out_d.ap()[row:row + P, :], o_t[:])

    nc.compile()
    return nc


def _prep_weights(Wq16, Wk16, Wv16):
    """Pre-arrange weights into SBUF tile layouts (shared by all cores)."""
    wq = np.ascontiguousarray(
        Wq16.reshape(DC, P, DC, P).transpose(1, 2, 0, 3))   # [p, ec, dc, e]
    wk = np.ascontiguousarray(
        Wk16.reshape(DC, P, DC, P).transpose(1, 2, 0, 3))  # [p, ec, dc, e]
    wv = np.ascontiguousarray(Wv16.reshape(DC, P, D).swapaxes(0, 1))
    return wq, wk, wv


def _prep_core_inputs(xT16, wq, wk, wv, b, h):
    """Host-side shard prep for core (batch b, half h)."""
    if h == 0:
        slots = (np.arange(1536, 2048), np.arange(0, 512))
    else:
        slots = (np.arange(1024, 1536), np.arange(512, 1024))
    tq = np.concatenate(slots)

    xTb = xT16[b]                                          # [D, T] fp16
    xq = np.ascontiguousarray(
        xTb[:, tq].reshape(DC, P, NSUP, QSUP).transpose(2, 1, 0, 3))
    xkv = np.ascontiguousarray(
        xTb[:, h * HT:(h + 1) * HT].reshape(DC, P, 2, QSUP).transpose(2, 1, 0, 3))

    masks = np.empty((P, NMSK, QSUP), dtype=np.float16)
    for i, (s, kt) in enumerate(MASKED):
        kidx = np.arange(kt * P, (kt + 1) * P).reshape(P, 1)
        tqs = tq[s * QSUP:(s + 1) * QSUP].reshape(1, QSUP)
        masks[:, i, :] = np.where(kidx <= tqs, 0.0, MASK_NEG).astype(np.float16)

    return {
        "xq": xq, "xkv": xkv, "wq": wq, "wk": wk, "wv": wv, "msk": masks,
    }, tq


def kernel(x, Wq, Wk, Wv):
    global last_exec_time_ns
    x = np.asarray(x, dtype=np.float32)
    assert x.shape == (B, T, D)

    if "nc" not in _CACHE:
        _CACHE["nc"] = _build_program()
    nc = _CACHE["nc"]

    xT16 = np.ascontiguousarray(
        x.transpose(0, 2, 1)).astype(np.float16)           # [B, D, T]
    wq, wk, wv = _prep_weights(
        np.asarray(Wq, dtype=np.float16),
        np.asarray(Wk, dtype=np.float16),
        np.asarray(Wv, dtype=np.float16))

    in_maps = []
    row_maps = []
    for c in range(8):
        im, tq = _prep_core_inputs(xT16, wq, wk, wv, c // 2, c % 2)
        in_maps.append(im)
        row_maps.append(tq)

    trace = bool(os.environ.get("BASS_KERNEL_TRACE"))
    kw = {}
    if trace:
        kw = {"trace": True, "tmpdir": os.environ.get(
            "BASS_KERNEL_TRACE_DIR", "/tmp/kernel_trace")}
    res = run_bass_kernel_spmd(nc, in_maps, core_ids=list(range(8)), **kw)
    if trace:
        last_exec_time_ns = res.exec_time_ns

    out = np.empty((B, T, D), dtype=np.float32)
    for c in range(8):
        out[c // 2, row_maps[c]] = res.results[c]["out"].astype(np.float32)
    return out


# revision 26
# speedup vs baseline: 1.1192x; 1.1192x over previous
"""Causal single-head attention on 8 Trainium2 NeuronCores.

Problem: x [4, 2048, 1024] f32; Wq/Wk/Wv [1024, 1024] f32.
  q,k,v = x@W*; out = softmax(causal(q k^T / sqrt(d))) @ v.

Sharding: 8 cores = 4 batches x 2 query-halves. Causal load balancing via
"fold" assignment of 512-query blocks: core (b, 0) takes query blocks
{3, 0} of its batch, core (b, 1) takes {2, 1}. Each core owns two
512-query "supers" whose key-prefix lengths are padded to the
compile-time slot shape (16, 8) x 128-key tiles; the pad region is
neutralized by an additive -60000 mask (host-built, tiny) so a single
program serves all cores (SPMD). Only the 16 tile-slots that are masked
on at least one core get the mask-add (in-place in PSUM); the other 8
exp straight out of the matmul accumulator.

The k/v projections are split across each core pair: every core projects
only its half of the batch tokens and the halves are exchanged with a
pair-wise AllGather through DRAM bounce buffers. Schedule: kproj ->
start kT gather -> vproj -> start v gather -> qproj s0 -> qproj s1 ->
scores s0 -> scores s1 -> AV s1 -> AV s0, so both collectives are
covered by ~55us of projection work and the PE never waits on them.
AV does the small super first: its 8 kt tiles only read v rows 0..1023
(the first AllGather half), buying time for the second half to land.

Post-collective redistribution is spread across the sync and gpsimd DMA
queues (a single queue serializes the chunks: the first one would
complete ~18us after issue instead of ~5), and the v collective is held
back until the kT chunks have drained (witness copies gate v_in's
staging DMA): collective fabric traffic starves local DMA rings, so any
chunk still in flight when a collective starts finishes only after it.

Precision: projections and att@v run in fp16 (1 PE cycle/row; fp32 is
4x slower); qT/kT are quantized to fp8e4m3 after the fp16 projection
and the scores matmuls use fp8 DoubleRow perf mode (2 contraction
subtiles per pass, ~1.8x fp16 throughput, and it halves the kT
exchange/redistribution bytes). Measured end-to-end rel err 1.18e-2
against the 2e-2 budget; accumulation stays fp32 in PSUM throughout.
Output is fp16, converted to f32 on the host (~1e-4 additional).

SBUF is tight, so the exp(scores) tiles alias scratch that is dead by
then: e(super 0) lives in the two xkv staging tiles and e(super 1) in
the vproj staging tile.

Per-core dataflow (all big matmuls N=512):
  kT-half[e, tok] fp8, v-half[tok, e] fp16 -> AllGather within pair
  qT[e, q] = Wq^T x_q^T fp16 -> fp8        (per super)
  scores S^T[k, q] = kT-block^T qT         (fp8 DoubleRow, 4 passes)
  E = exp((S^T [+ mask]) / 32)             (ACT, fp16 out)
  out[q, e] = (E^T v) / (E^T 1)            (denominator via N=2
                                            ones-matmul, fp16)

Softmax max-subtraction is skipped deliberately: logits*scale are
bounded (|s|/32 < ~2.5), so exp is well-conditioned.
"""

import os
import sys

sys.path.insert(0, "/opt/trn_rl_repo")

from contextlib import ExitStack

import numpy as np

import concourse.bass as bass  # noqa: F401
import concourse.tile as tile
from concourse import bacc, mybir
from concourse.bass_utils import run_bass_kernel_spmd

B, T, D = 4, 2048, 1024
P = 128                 # partitions
DC = D // P             # 8 contraction chunks
QSUP = 512              # queries per super
NSUP = 2                # supers per core
NQ = QSUP * NSUP        # 1024 queries per core
SLOT_KT = (16, 8)       # 128-key tiles per super slot (compile-time, all cores)
NKT = sum(SLOT_KT)      # 24
HT = T // 2             # 1024 tokens projected per core (half of the pair)
HCH = HT // P           # 8 token chunks per half
TCH = T // P            # 16 key/value 128-token chunks
SCALE = 1.0 / 32.0      # 1/sqrt(D)
MASK_NEG = -60000.0     # representable in fp16; exp((s+m)/32) == 0

# tile-slots that get the additive mask (union over both cores of a pair):
# slot A (16 tiles): tiles 8..15; slot B (8 tiles): tiles 0..7.
MASKED = [(0, kt) for kt in range(8, 16)] + [(1, kt) for kt in range(8)]
NMSK = len(MASKED)      # 16

F16 = mybir.dt.float16
F32 = mybir.dt.float32
F8 = mybir.dt.float8e4

_CACHE = {}

last_exec_time_ns = None  # set when BASS_KERNEL_TRACE=1


def _build_program():
    nc = bacc.Bacc("TRN2", target_bir_lowering=False, debug=False, num_devices=8)

    xq_d = nc.dram_tensor("xq", [NSUP, P, DC, QSUP], F16, kind="ExternalInput")
    xkv_d = nc.dram_tensor("xkv", [2, P, DC, QSUP], F16, kind="ExternalInput")
    wq_d = nc.dram_tensor("wq", [P, DC, DC, P], F16, kind="ExternalInput")
    wk_d = nc.dram_tensor("wk", [P, DC, DC, P], F16, kind="ExternalInput")
    wv_d = nc.dram_tensor("wv", [P, DC, D], F16, kind="ExternalInput")
    msk_d = nc.dram_tensor("msk", [P, NMSK, QSUP], F16, kind="ExternalInput")
    out_d = nc.dram_tensor("out", [NQ, D], F16, kind="ExternalOutput")

    with tile.TileContext(nc) as tc, ExitStack() as stack:
        p_wq = stack.enter_context(tc.tile_pool(name="wq", bufs=1))
        p_xq = stack.enter_context(tc.tile_pool(name="xq", bufs=1))
        p_kt = stack.enter_context(tc.tile_pool(name="kt", bufs=1))
        p_v = stack.enter_context(tc.tile_pool(name="v", bufs=1))
        p_qt = stack.enter_context(tc.tile_pool(name="qt", bufs=1))
        p_misc = stack.enter_context(tc.tile_pool(name="misc", bufs=1))
        p_dram = stack.enter_context(tc.tile_pool(name="dram", bufs=1, space="DRAM"))
        p_wk = stack.enter_context(tc.tile_pool(name="wk", bufs=1))
        p_wv = stack.enter_context(tc.tile_pool(name="wv", bufs=1))
        p_xkv = stack.enter_context(tc.tile_pool(name="xkv", bufs=1))
        p_half = stack.enter_context(tc.tile_pool(name="half", bufs=1))
        p_m = stack.enter_context(tc.tile_pool(name="msk", bufs=1))
        p_out = stack.enter_context(tc.tile_pool(name="outp", bufs=2))
        ps512 = stack.enter_context(tc.tile_pool(name="ps512", bufs=3, space="PSUM"))
        psav = stack.enter_context(tc.tile_pool(name="psav", bufs=2, space="PSUM"))
        psd = stack.enter_context(tc.tile_pool(name="psd", bufs=1, space="PSUM"))

        # ---- persistent tensors ----
        kt_t = p_kt.tile([P, DC, T], F8)            # k^T  [e, tok] fp8
        v_t = p_v.tile([P, TCH, D], F16)            # v    [tok, e]
        # staging scratch shared by kproj ([P, ec, tok]) then vproj
        # ([P, tok-chunk, e]) then e(super 1); all roles fit [P, 8, 1024]
        half_t = p_half.tile([P, 8, 1024], F16)
        kt8_t = p_half.tile([P, DC, HT], F8, tag="kt8")   # kproj fp8 staging
        msk_t = p_m.tile([P, NMSK, QSUP], F16)

        ones_t = p_misc.tile([P, 2], F16, tag="ones")
        nc.gpsimd.memset(ones_t[:], 1.0)

        # ---- need-ordered input loads ----
        # the first kproj accumulation group needs wk[0] + xkv[0]: fan
        # those across all three DMA queues so they land in parallel,
        # then stream the rest as few large transfers as possible
        w0 = p_wk.tile([P, DC, P], F16, tag="wk0")
        nc.sync.dma_start(w0[:], wk_d.ap()[:, 0])
        xc0 = p_xkv.tile([P, DC, QSUP], F16, tag="xkv0")
        nc.scalar.dma_start(xc0[:, 0:4, :], xkv_d.ap()[0][:, 0:4, :])
        nc.gpsimd.dma_start(xc0[:, 4:8, :], xkv_d.ap()[0][:, 4:8, :])
        xkv_c = [xc0]
        wk_all = p_wk.tile([P, DC - 1, DC, P], F16, tag="wkall")
        for ec in range(1, DC):
            nc.sync.dma_start(wk_all[:, ec - 1], wk_d.ap()[:, ec])
        wk_c = [w0] + [wk_all[:, ec - 1] for ec in range(1, DC)]
        xc1 = p_xkv.tile([P, DC, QSUP], F16, tag="xkv1")
        nc.gpsimd.dma_start(xc1[:], xkv_d.ap()[1])
        xkv_c.append(xc1)
        # wv alone behind xkv0a on the scalar ring (vproj needs it ~50us
        # in); the bulky q-side transfers ride sync behind the wk stream
        wv_t = p_wv.tile([P, DC, D], F16)
        nc.scalar.dma_start(wv_t[:], wv_d.ap())
        xq_tiles = []
        xq_q = [nc.sync, nc.gpsimd]   # super 1 inputs ride the gpsimd ring
        for s in range(NSUP):
            xq_t = p_xq.tile([P, DC, QSUP], F16, tag=f"xq{s}")
            for dc in range(DC):
                xq_q[s].dma_start(xq_t[:, dc], xq_d.ap()[s][:, dc])
            xq_tiles.append([xq_t[:, dc] for dc in range(DC)])
        wq_all = p_wq.tile([P, DC, DC, P], F16, tag="wqall")
        for ec in range(DC):
            nc.gpsimd.dma_start(wq_all[:, ec], wq_d.ap()[:, ec])
        wq_tiles = [wq_all[:, ec] for ec in range(DC)]
        nc.gpsimd.dma_start(msk_t[:], msk_d.ap())

        # ---- P1a: kT for own half -> pair AllGather ----
        # ec-outer: each wk chunk feeds two accumulation groups before the
        # next chunk is needed, doubling the DMA deadline per chunk
        for ec in range(DC):
            for kt2 in range(HT // QSUP):
                acc = ps512.tile([P, QSUP], F32, tag="ps512")
                for dc in range(DC):
                    nc.tensor.matmul(
                        acc[:], wk_c[ec][:, dc, :],
                        xkv_c[kt2][:, dc, :],
                        start=(dc == 0), stop=(dc == DC - 1))
                nc.scalar.copy(
                    kt8_t[:, ec, kt2 * QSUP:(kt2 + 1) * QSUP], acc[:])
        kt_in = p_dram.tile([P, DC, HT], F8, tag="kt_in")
        kt_out = p_dram.tile([2, P, DC, HT], F8, tag="kt_out")
        nc.sync.dma_start(kt_in[:], kt8_t[:])
        nc.gpsimd.collective_compute(
            "AllGather", mybir.AluOpType.bypass,
            replica_groups=[[0, 1], [2, 3], [4, 5], [6, 7]],
            ins=[kt_in.opt()], outs=[kt_out.opt()])

        # ---- P1b: v for own half -> pair AllGather ----
        for tk in range(HCH):
            for eh in range(2):
                acc = ps512.tile([P, QSUP], F32, tag="ps512")
                for dc in range(DC):
                    nc.tensor.matmul(
                        acc[:],
                        xkv_c[tk // 4][:, dc, (tk % 4) * P:(tk % 4 + 1) * P],
                        wv_t[:, dc, eh * QSUP:(eh + 1) * QSUP],
                        start=(dc == 0), stop=(dc == DC - 1))
                nc.vector.tensor_copy(
                    half_t[:, tk, eh * QSUP:(eh + 1) * QSUP], acc[:])
        v_in = p_dram.tile([P, HCH, D], F16, tag="v_in")
        v_out = p_dram.tile([2, P, HCH, D], F16, tag="v_out")

        # ---- kT distribution: 512-key chunks spread over sync+gpsimd ----
        dist_q = [nc.sync, nc.sync, nc.gpsimd, nc.gpsimd]
        for h in range(2):
            for c2 in range(2):
                dist_q[h * 2 + c2].dma_start(
                    kt_t[:, :, h * HT + c2 * QSUP:h * HT + (c2 + 1) * QSUP],
                    kt_out[h][:, :, c2 * QSUP:(c2 + 1) * QSUP])
        # hold the v collective until ALL FOUR kt distribution chunks have
        # drained: collective fabric traffic starves the local DMA rings,
        # so any chunk still in flight when it starts completes only after
        # it ends. The CC-core pulls collectives as soon as their INPUT is
        # ready (queue order alone cannot delay it), so v_in's staging DMA
        # itself is gated behind per-chunk witness copies.
        kt_rdy = p_misc.tile([P, 4], F16, tag="kt_rdy")
        for c4 in range(4):
            nc.gpsimd.tensor_copy(
                kt_rdy[:, c4:c4 + 1],
                kt_t[:, DC - 1, (c4 + 1) * QSUP - 1:(c4 + 1) * QSUP])
        nc.gpsimd.dma_start(v_in[:], half_t[:])

        nc.gpsimd.collective_compute(
            "AllGather", mybir.AluOpType.bypass,
            replica_groups=[[0, 1], [2, 3], [4, 5], [6, 7]],
            ins=[v_in.opt()], outs=[v_out.opt()])
        for h in range(2):
            for c2 in range(2):
                dist_q[h * 2 + c2].dma_start(
                    v_t[:, h * HCH + c2 * 4:h * HCH + (c2 + 1) * 4, :],
                    v_out[h][:, c2 * 4:(c2 + 1) * 4, :])

        # ---- q projections (cover the AllGathers) ----
        qt_s = []
        for s in range(NSUP):
            xq_c = xq_tiles[s]
            qt_t = p_qt.tile([P, DC, QSUP], F8, tag=f"qt{s}")
            for ec in range(DC):
                acc = ps512.tile([P, QSUP], F32, tag="ps512")
                for dc in range(DC):
                    nc.tensor.matmul(acc[:], wq_tiles[ec][:, dc, :],
                                     xq_c[dc][:],
                                     start=(dc == 0), stop=(dc == DC - 1))
                nc.scalar.copy(qt_t[:, ec, :], acc[:])
            qt_s.append(qt_t)

        # ---- P2a: scores + exp for both supers ----
        # e(super 0) aliases the dead xkv staging tiles, e(super 1) the
        # kproj/vproj scratch
        def e_slice(s, kt, cols=slice(0, QSUP)):
            if s == 0:
                return xkv_c[kt // DC][:, kt % DC, cols]
            return half_t[:, kt, cols]

        mask_i = {(s, kt): i for i, (s, kt) in enumerate(MASKED)}
        for s in range(NSUP):
            nkt = SLOT_KT[s]
            qt = qt_s[s]
            for kt in range(nkt):
                acc = ps512.tile([P, QSUP], F32, tag="ps512")
                for e2 in range(DC // 2):
                    nc.tensor.matmul(
                        acc[:],
                        kt_t[:, 2 * e2:2 * e2 + 2, kt * P:(kt + 1) * P],
                        qt[:, 2 * e2:2 * e2 + 2, :],
                        start=(e2 == 0), stop=(e2 == DC // 2 - 1),
                        perf_mode=mybir.MatmulPerfMode.DoubleRow)
                mi = mask_i.get((s, kt))
                if mi is not None:
                    nc.vector.tensor_add(acc[:], acc[:], msk_t[:, mi, :])
                nc.scalar.activation(e_slice(s, kt), acc[:],
                                     mybir.ActivationFunctionType.Exp,
                                     scale=SCALE)

        # ---- P2b: att @ v, normalize, store ----
        d_acc = psd.tile([P, 4, 2], F32)
        # super 1 first: its 8 kt tiles only touch v rows 0..1023, which
        # arrive in the first v_out half; super 0's tail tiles need the
        # second half, which lands while super 1 is being computed
        for s in (1, 0):
            nkt = SLOT_KT[s]
            for qs in range(4):
                o_acc = psav.tile([P, D], F32, tag="av")
                for kt in range(nkt):
                    lhs = e_slice(s, kt, slice(qs * P, (qs + 1) * P))
                    nc.tensor.matmul(o_acc[:, 0:QSUP], lhs,
                                     v_t[:, kt, 0:QSUP],
                                     start=(kt == 0),
                                     stop=(kt == nkt - 1))
                    nc.tensor.matmul(o_acc[:, QSUP:D], lhs,
                                     v_t[:, kt, QSUP:D],
                                     start=(kt == 0),
                                     stop=(kt == nkt - 1))
                    nc.tensor.matmul(d_acc[:, qs, :], lhs, ones_t[:],
                                     start=(kt == 0),
                                     stop=(kt == nkt - 1))
                dinv = p_misc.tile([P, 1], F32, tag=f"dinv{s}{qs}")
                nc.vector.reciprocal(dinv[:], d_acc[:, qs, 0:1])
                o_t = p_out.tile([P, D], F16, tag="o")
                nc.vector.tensor_scalar_mul(o_t[:], o_acc[:], dinv[:])
                row = s * QSUP + qs * P
                nc.sync.dma_start(out_d.ap()[row:row + P, :], o_t[:])

    nc.compile()
    return nc


def _prep_weights(Wq16, Wk16, Wv16):
    """Pre-arrange weights into SBUF tile layouts (shared by all cores)."""
    wq = np.ascontiguousarray(
        Wq16.reshape(DC, P, DC, P).transpose(1, 2, 0, 3))   # [p, ec, dc, e]
    wk = np.ascontiguousarray(
        Wk16.reshape(DC, P, DC, P).transpose(1, 2, 0, 3))  # [p, ec, dc, e]
    wv = np.ascontiguousarray(Wv16.reshape(DC, P, D).swapaxes(0, 1))
    return wq, wk, wv


def _prep_core_inputs(xT16, wq, wk, wv, b, h):
    """Host-side shard prep for core (batch b, half h)."""
    if h == 0:
        slots = (np.arange(1536, 2048), np.arange(0, 512))
    else:
        slots = (np.arange(1024, 1536), np.arange(512, 1024))
    tq = np.concatenate(slots)

    xTb = xT16[b]                                          # [D, T] fp16
    xq = np.ascontiguousarray(
        xTb[:, tq].reshape(DC, P, NSUP, QSUP).transpose(2, 1, 0, 3))
    xkv = np.ascontiguousarray(
        xTb[:, h * HT:(h + 1) * HT].reshape(DC, P, 2, QSUP).transpose(2, 1, 0, 3))

    masks = np.empty((P, NMSK, QSUP), dtype=np.float16)
    for i, (s, kt) in enumerate(MASKED):
        kidx = np.arange(kt * P, (kt + 1) * P).reshape(P, 1)
        tqs = tq[s * QSUP:(s + 1) * QSUP].reshape(1, QSUP)
        masks[:, i, :] = np.where(kidx <= tqs, 0.0, MASK_NEG).astype(np.float16)

    return {
        "xq": xq, "xkv": xkv, "wq": wq, "wk": wk, "wv": wv, "msk": masks,
    }, tq


def kernel(x, Wq, Wk, Wv):
    global last_exec_time_ns
    x = np.asarray(x, dtype=np.float32)
    assert x.shape == (B, T, D)

    if "nc" not in _CACHE:
        _CACHE["nc"] = _build_program()
    nc = _CACHE["nc"]

    xT16 = np.ascontiguousarray(
        x.transpose(0, 2, 1)).astype(np.float16)           # [B, D, T]
    wq, wk, wv = _prep_weights(
        np.asarray(Wq, dtype=np.float16),
        np.asarray(Wk, dtype=np.float16),
        np.asarray(Wv, dtype=np.float16))

    in_maps = []
    row_maps = []
    for c in range(8):
        im, tq = _prep_core_inputs(xT16, wq, wk, wv, c // 2, c % 2)
        in_maps.append(im)
        row_maps.append(tq)

    trace = bool(os.environ.get("BASS_KERNEL_TRACE"))
    kw = {}
    if trace:
        kw = {"trace": True, "tmpdir": os.environ.get(
            "BASS_KERNEL_TRACE_DIR", "/tmp/kernel_trace")}
    res = run_bass_kernel_spmd(nc, in_maps, core_ids=list(range(8)), **kw)
    if trace:
        last_exec_time_ns = res.exec_time_ns

    out = np.empty((B, T, D), dtype=np.float32)
    for c in range(8):
        out[c // 2, row_maps[c]] = res.results[c]["out"].astype(np.float32)
    return out
